# revision 7
# baseline (speedup 1.0000x reference)
"""Bahdanau (additive) attention kernel for Trainium2, SPMD over 8 NeuronCores.

Problem shapes (hardcoded): B=8, Tq=Tv=128, D=512, U=1024, fp32.
Sharding: data-parallel over batch -- one batch element per core; W1/W2/scale
replicated.

Per-core algorithm:
  wq = q @ W1            [Tq, U]   (kept transposed: [u_part, t])
  wk = v @ W2            [Tv, U]   (kept transposed: [u_part, s])
  scores[t, s] = sum_u scale[u] * tanh(wq[t, u] + wk[s, u])
     - broadcast-add on DVE (tensor_scalar, per-partition scalar = wq column)
     - tanh on ACT engine (big fused tiles)
     - reduction over u via PE matmuls (lhsT = tanh tile [u, s], rhs = scale
       column [u, 1]) accumulating into a PSUM tile laid out [s, t]
  softmax over s (key axis) with max-subtraction, on [t, s] layout
  context = attn @ v via one PE matmul (lhsT = attn^T [s, t], rhs = v [s, d])
"""

import numpy as np

B, Tq, Tv, D, U = 8, 128, 128, 512, 1024
P = 128
UC = U // P  # 8 u-chunks
DC = D // P  # 4 d-chunks
G_T = 8      # query rows per tanh group
NCORES = 8

_CACHE = {}


def _build_bass():
    import concourse.bass as bass
    import concourse.mybir as mybir
    from concourse import bacc
    import concourse.tile as tile
    from concourse.masks import make_identity

    f32 = mybir.dt.float32
    bf16 = mybir.dt.bfloat16
    u8 = mybir.dt.uint8
    AF = mybir.ActivationFunctionType

    nc = bacc.Bacc()

    q_d = nc.declare_dram_parameter("q", [Tq, D], f32, isOutput=False)
    v_d = nc.declare_dram_parameter("v", [Tv, D], f32, isOutput=False)
    w1_d = nc.declare_dram_parameter("w1", [D, U], f32, isOutput=False)
    w2_d = nc.declare_dram_parameter("w2", [D, U], f32, isOutput=False)
    sc_d = nc.declare_dram_parameter("scale_pc", [P, UC], f32, isOutput=False)
    mk_d = nc.declare_dram_parameter("mask_p", [P, 1], u8, isOutput=False)
    ctx_d = nc.declare_dram_parameter("ctx", [Tq, D], f32, isOutput=True)
    attn_d = nc.declare_dram_parameter("attn", [Tq, Tv], f32, isOutput=True)

    with tile.TileContext(nc) as tc:
        with (
            tc.tile_pool(name="const", bufs=1) as const_pool,
            tc.tile_pool(name="w", bufs=1) as w_pool,
            tc.tile_pool(name="sb", bufs=1) as sb_pool,
            tc.tile_pool(name="stage", bufs=3) as stage_pool,
            tc.tile_pool(name="mt", bufs=3) as m_pool,
            tc.tile_pool(name="misc", bufs=2) as misc_pool,
            tc.tile_pool(name="ps", bufs=2, space="PSUM") as ps_pool,
            tc.tile_pool(name="accps", bufs=1, space="PSUM") as acc_ps_pool,
        ):
            ident = const_pool.tile([P, P], f32)
            make_identity(nc, ident)

            # Warm the ACT table set (exp_and_others contains tanh+exp) ASAP
            # so the ~2.7us table load overlaps the input DMAs.
            dummy = const_pool.tile([P, 1], f32)
            nc.vector.memset(dummy, 0.0)
            nc.scalar.activation(dummy, dummy, AF.Tanh)

            scale_sb = const_pool.tile([P, UC], f32)
            nc.sync.dma_start(scale_sb[:], sc_d[:])
            scale_bf = const_pool.tile([P, UC], bf16)
            nc.vector.tensor_copy(scale_bf[:], scale_sb[:])
            mask_u8 = const_pool.tile([P, 1], u8)
            nc.sync.dma_start(mask_u8[:], mk_d[:])
            # mask_neg[s] = (mask[s] - 1) * 1e9  -> 0 where valid, -1e9 where masked
            mask_neg = const_pool.tile([P, 1], f32)
            nc.vector.tensor_scalar(
                out=mask_neg, in0=mask_u8, scalar1=1.0, scalar2=1e9,
                op0=mybir.AluOpType.subtract, op1=mybir.AluOpType.mult,
            )

            q_sb = sb_pool.tile([P, D], f32)
            nc.sync.dma_start(q_sb[:], q_d[:])
            v_sb = sb_pool.tile([P, D], f32)
            nc.sync.dma_start(v_sb[:], v_d[:])
            w1_sb = w_pool.tile([P, DC, U], f32)
            w2_sb = w_pool.tile([P, DC, U], f32)
            w1_r = w1_d.rearrange("(dc p) u -> p dc u", p=P)
            w2_r = w2_d.rearrange("(dc p) u -> p dc u", p=P)
            H = U // 2
            nc.sync.dma_start(w2_sb[:, :, :H], w2_r[:, :, :H])
            nc.sync.dma_start(w1_sb[:, :, :H], w1_r[:, :, :H])
            nc.sync.dma_start(w2_sb[:, :, H:], w2_r[:, :, H:])
            nc.sync.dma_start(w1_sb[:, :, H:], w1_r[:, :, H:])

            # Transpose q, v to [d_part, t/s] chunks for the projection matmuls.
            qT = sb_pool.tile([P, DC, P], f32)
            vT = sb_pool.tile([P, DC, P], f32)
            for src, dst in ((q_sb, qT), (v_sb, vT)):
                for dc in range(DC):
                    tps = ps_pool.tile([P, P], f32, tag="tp")
                    nc.tensor.transpose(tps, src[:, dc * P:(dc + 1) * P], ident)
                    nc.scalar.copy(dst[:, dc, :], tps)

            # Projections: wq[p, c, t] = sum_d W1[d, c*128+p] * q[t, d]
            wq = sb_pool.tile([P, UC, P], f32)
            wk = sb_pool.tile([P, UC, P], bf16)
            for c in range(UC):
                for wsrc, xT, dst in ((w2_sb, vT, wk), (w1_sb, qT, wq)):
                    pps = ps_pool.tile([P, P], f32, tag="proj")
                    for dc in range(DC):
                        nc.tensor.matmul(
                            pps,
                            lhsT=wsrc[:, dc, c * P:(c + 1) * P],
                            rhs=xT[:, dc, :],
                            start=(dc == 0),
                            stop=(dc == DC - 1),
                        )
                    nc.scalar.copy(dst[:, c, :], pps)

            # Main loop: scores_ps[s, t] accumulated over u-chunks.
            scores_ps = acc_ps_pool.tile([P, Tq], f32, tag="scores")
            for g in range(Tq // G_T):
                stage = stage_pool.tile([P, G_T * UC * P], bf16, tag="stage")
                mt = m_pool.tile([P, G_T * UC * P], bf16, tag="mt")
                for tl in range(G_T):
                    t = g * G_T + tl
                    for c in range(UC):
                        off = (tl * UC + c) * P
                        eng = nc.gpsimd if (t * UC + c) % 4 == 3 else nc.vector
                        eng.tensor_scalar_add(
                            out=stage[:, off:off + P],
                            in0=wk[:, c, :],
                            scalar1=wq[:, c, t:t + 1],
                        )
                nc.scalar.activation(mt, stage, AF.Tanh)
                for tl in range(G_T):
                    t = g * G_T + tl
                    for c in range(UC):
                        off = (tl * UC + c) * P
                        nc.tensor.matmul(
                            scores_ps[:, t:t + 1],
                            lhsT=mt[:, off:off + P],
                            rhs=scale_bf[:, c:c + 1],
                            start=(c == 0),
                            stop=(c == UC - 1),
                        )

            # Mask + evacuate psum (scores laid out [s, t]).
            scores_sb = misc_pool.tile([P, Tq], f32)
            nc.vector.tensor_scalar_add(scores_sb, scores_ps, mask_neg)

            # Transpose to [t, s] for free-axis softmax.
            tsp = ps_pool.tile([P, P], f32, tag="tp")
            nc.tensor.transpose(tsp, scores_sb, ident)
            neg_mx = misc_pool.tile([P, 1], f32)
            nc.vector.reduce_max(neg_mx, tsp, axis=mybir.AxisListType.X, negate=True)
            p_sb = misc_pool.tile([P, Tv], f32)
            sum_sb = misc_pool.tile([P, 1], f32)
            nc.scalar.activation(p_sb, tsp, AF.Exp, bias=neg_mx, accum_out=sum_sb)
            rsum = misc_pool.tile([P, 1], f32)
            nc.vector.reciprocal(rsum, sum_sb)
            attn_sb = misc_pool.tile([P, Tv], f32)
            nc.vector.tensor_scalar_mul(attn_sb, p_sb, rsum)
            nc.sync.dma_start(attn_d[:], attn_sb[:])

            # context = attn @ v : lhsT = attn^T [s, t], rhs = v [s, d]
            atp = ps_pool.tile([P, P], f32, tag="tp")
            nc.tensor.transpose(atp, attn_sb, ident)
            attnT = misc_pool.tile([P, Tq], f32)
            nc.scalar.copy(attnT, atp)
            ctx_ps = acc_ps_pool.tile([P, D], f32, tag="ctx")
            nc.tensor.matmul(ctx_ps, lhsT=attnT, rhs=v_sb, start=True, stop=True)
            ctx_sb = misc_pool.tile([P, D], f32)
            nc.vector.tensor_copy(ctx_sb, ctx_ps)
            nc.sync.dma_start(ctx_d[:], ctx_sb[:])

    return nc


def get_bass():
    if "nc" not in _CACHE:
        nc = _build_bass()
        # Run the bacc passes (wait splitting, register allocation, act table
        # loads) and freeze before the PJRT lowering serializes the module.
        nc.finalize()
        _CACHE["nc"] = nc
    return _CACHE["nc"]


def make_in_maps(query, value, mask, W1, W2, scale):
    query = np.ascontiguousarray(np.asarray(query, dtype=np.float32))
    value = np.ascontiguousarray(np.asarray(value, dtype=np.float32))
    mask = np.asarray(mask)
    W1 = np.ascontiguousarray(np.asarray(W1, dtype=np.float32))
    W2 = np.ascontiguousarray(np.asarray(W2, dtype=np.float32))
    scale = np.asarray(scale, dtype=np.float32)
    # scale_pc[p, c] = scale[c*128 + p]
    scale_pc = np.ascontiguousarray(scale.reshape(UC, P).T)
    in_maps = []
    for b in range(B):
        in_maps.append({
            "q": np.ascontiguousarray(query[b]),
            "v": np.ascontiguousarray(value[b]),
            "w1": W1,
            "w2": W2,
            "scale_pc": scale_pc,
            "mask_p": np.ascontiguousarray(
                mask[b].astype(np.uint8).reshape(P, 1)),
        })
    return in_maps


def run(query, value, mask, W1, W2, scale, trace=False):
    from concourse.bass_utils import run_bass_kernel_spmd

    nc = get_bass()
    in_maps = make_in_maps(query, value, mask, W1, W2, scale)
    res = run_bass_kernel_spmd(nc, in_maps, list(range(NCORES)), trace=trace)
    ctx = np.stack([res.results[b]["ctx"] for b in range(B)])
    attn = np.stack([res.results[b]["attn"] for b in range(B)])
    return (ctx, attn), res


def kernel(**inputs):
    (ctx, attn), _ = run(
        inputs["query"], inputs["value"], inputs["mask"],
        inputs["W1"], inputs["W2"], inputs["scale"],
    )
    return ctx, attn


# revision 8
# speedup vs baseline: 2.6119x; 2.6119x over previous
"""Bahdanau (additive) attention kernel for Trainium2, SPMD over 8 NeuronCores.

Problem shapes (hardcoded): B=8, Tq=Tv=128, D=512, U=1024, fp32.
Sharding: data-parallel over batch -- one batch element per core; W1/W2/scale
replicated.

Per-core algorithm:
  wq = q @ W1            [Tq, U]   (kept transposed: [u_part, t])
  wk = v @ W2            [Tv, U]   (kept transposed: [u_part, s])
  scores[t, s] = sum_u scale[u] * tanh(wq[t, u] + wk[s, u])
     - broadcast-add on DVE (tensor_scalar, per-partition scalar = wq column)
     - tanh on ACT engine (big fused tiles)
     - reduction over u via PE matmuls (lhsT = tanh tile [u, s], rhs = scale
       column [u, 1]) accumulating into a PSUM tile laid out [s, t]
  softmax over s (key axis) with max-subtraction, on [t, s] layout
  context = attn @ v via one PE matmul (lhsT = attn^T [s, t], rhs = v [s, d])
"""

import numpy as np

B, Tq, Tv, D, U = 8, 128, 128, 512, 1024
P = 128
UC = U // P  # 8 u-chunks
DC = D // P  # 4 d-chunks
G_T = 8      # query rows per tanh group
NCORES = 8

_CACHE = {}


def _build_bass():
    import concourse.bass as bass
    import concourse.mybir as mybir
    from concourse import bacc
    import concourse.tile as tile
    from concourse.masks import make_identity

    f32 = mybir.dt.float32
    bf16 = mybir.dt.bfloat16
    u8 = mybir.dt.uint8
    AF = mybir.ActivationFunctionType

    nc = bacc.Bacc()

    q_d = nc.declare_dram_parameter("q", [Tq, D], f32, isOutput=False)
    v_d = nc.declare_dram_parameter("v", [Tv, D], f32, isOutput=False)
    w1_d = nc.declare_dram_parameter("w1", [D, U], f32, isOutput=False)
    w2_d = nc.declare_dram_parameter("w2", [D, U], f32, isOutput=False)
    sc_d = nc.declare_dram_parameter("scale_pc", [P, UC], f32, isOutput=False)
    mk_d = nc.declare_dram_parameter("mask_p", [P, 1], u8, isOutput=False)
    ctx_d = nc.declare_dram_parameter("ctx", [Tq, D], f32, isOutput=True)
    attn_d = nc.declare_dram_parameter("attn", [Tq, Tv], f32, isOutput=True)

    with tile.TileContext(nc) as tc:
        with (
            tc.tile_pool(name="const", bufs=1) as const_pool,
            tc.tile_pool(name="w", bufs=1) as w_pool,
            tc.tile_pool(name="sb", bufs=1) as sb_pool,
            tc.tile_pool(name="stage", bufs=3) as stage_pool,
            tc.tile_pool(name="mt", bufs=3) as m_pool,
            tc.tile_pool(name="misc", bufs=2) as misc_pool,
            tc.tile_pool(name="ps", bufs=2, space="PSUM") as ps_pool,
            tc.tile_pool(name="accps", bufs=1, space="PSUM") as acc_ps_pool,
        ):
            ident = const_pool.tile([P, P], f32)
            make_identity(nc, ident)

            # Warm the ACT table set (exp_and_others contains tanh+exp) ASAP
            # so the ~2.7us table load overlaps the input DMAs.
            dummy = const_pool.tile([P, 1], f32)
            nc.vector.memset(dummy, 0.0)
            nc.scalar.activation(dummy, dummy, AF.Tanh)

            scale_sb = const_pool.tile([P, UC], f32)
            nc.sync.dma_start(scale_sb[:], sc_d[:])
            scale_bf = const_pool.tile([P, UC], bf16)
            nc.vector.tensor_copy(scale_bf[:], scale_sb[:])
            mask_u8 = const_pool.tile([P, 1], u8)
            nc.sync.dma_start(mask_u8[:], mk_d[:])
            # mask_neg[s] = (mask[s] - 1) * 1e9  -> 0 where valid, -1e9 where masked
            mask_neg = const_pool.tile([P, 1], f32)
            nc.vector.tensor_scalar(
                out=mask_neg, in0=mask_u8, scalar1=1.0, scalar2=1e9,
                op0=mybir.AluOpType.subtract, op1=mybir.AluOpType.mult,
            )

            q_sb = sb_pool.tile([P, D], f32)
            nc.sync.dma_start(q_sb[:], q_d[:])
            v_sb = sb_pool.tile([P, D], f32)
            nc.sync.dma_start(v_sb[:], v_d[:])
            w1_sb = w_pool.tile([P, DC, U], f32)
            w2_sb = w_pool.tile([P, DC, U], f32)
            w1_r = w1_d.rearrange("(dc p) u -> p dc u", p=P)
            w2_r = w2_d.rearrange("(dc p) u -> p dc u", p=P)
            H = U // 2
            nc.sync.dma_start(w2_sb[:, :, :H], w2_r[:, :, :H])
            nc.sync.dma_start(w1_sb[:, :, :H], w1_r[:, :, :H])
            nc.sync.dma_start(w2_sb[:, :, H:], w2_r[:, :, H:])
            nc.sync.dma_start(w1_sb[:, :, H:], w1_r[:, :, H:])

            # Transpose q, v to [d_part, t/s] chunks for the projection matmuls.
            qT = sb_pool.tile([P, DC, P], f32)
            vT = sb_pool.tile([P, DC, P], f32)
            for src, dst in ((q_sb, qT), (v_sb, vT)):
                for dc in range(DC):
                    tps = ps_pool.tile([P, P], f32, tag="tp")
                    nc.tensor.transpose(tps, src[:, dc * P:(dc + 1) * P], ident)
                    nc.scalar.copy(dst[:, dc, :], tps)

            # Projections: wq[p, c, t] = sum_d W1[d, c*128+p] * q[t, d]
            wq = sb_pool.tile([P, UC, P], f32)
            wk = sb_pool.tile([P, UC, P], bf16)
            for c in range(UC):
                for wsrc, xT, dst in ((w2_sb, vT, wk), (w1_sb, qT, wq)):
                    pps = ps_pool.tile([P, P], f32, tag="proj")
                    for dc in range(DC):
                        nc.tensor.matmul(
                            pps,
                            lhsT=wsrc[:, dc, c * P:(c + 1) * P],
                            rhs=xT[:, dc, :],
                            start=(dc == 0),
                            stop=(dc == DC - 1),
                        )
                    nc.scalar.copy(dst[:, c, :], pps)

            # Main loop: scores_ps[s, t] accumulated over u-chunks.
            scores_ps = acc_ps_pool.tile([P, Tq], f32, tag="scores")
            for g in range(Tq // G_T):
                stage = stage_pool.tile([P, G_T * UC * P], bf16, tag="stage")
                mt = m_pool.tile([P, G_T * UC * P], bf16, tag="mt")
                for tl in range(G_T):
                    t = g * G_T + tl
                    for c in range(UC):
                        off = (tl * UC + c) * P
                        nc.vector.tensor_scalar_add(
                            out=stage[:, off:off + P],
                            in0=wk[:, c, :],
                            scalar1=wq[:, c, t:t + 1],
                        )
                nc.scalar.activation(mt, stage, AF.Tanh)
                for tl in range(G_T):
                    t = g * G_T + tl
                    for c in range(UC):
                        off = (tl * UC + c) * P
                        nc.tensor.matmul(
                            scores_ps[:, t:t + 1],
                            lhsT=mt[:, off:off + P],
                            rhs=scale_bf[:, c:c + 1],
                            start=(c == 0),
                            stop=(c == UC - 1),
                        )

            # Mask + evacuate psum (scores laid out [s, t]).
            scores_sb = misc_pool.tile([P, Tq], f32)
            nc.vector.tensor_scalar_add(scores_sb, scores_ps, mask_neg)

            # Transpose to [t, s] for free-axis softmax.
            tsp = ps_pool.tile([P, P], f32, tag="tp")
            nc.tensor.transpose(tsp, scores_sb, ident)
            neg_mx = misc_pool.tile([P, 1], f32)
            nc.vector.reduce_max(neg_mx, tsp, axis=mybir.AxisListType.X, negate=True)
            p_sb = misc_pool.tile([P, Tv], f32)
            sum_sb = misc_pool.tile([P, 1], f32)
            nc.scalar.activation(p_sb, tsp, AF.Exp, bias=neg_mx, accum_out=sum_sb)
            rsum = misc_pool.tile([P, 1], f32)
            nc.vector.reciprocal(rsum, sum_sb)
            attn_sb = misc_pool.tile([P, Tv], f32)
            nc.vector.tensor_scalar_mul(attn_sb, p_sb, rsum)
            nc.sync.dma_start(attn_d[:], attn_sb[:])

            # context = attn @ v : lhsT = attn^T [s, t], rhs = v [s, d]
            atp = ps_pool.tile([P, P], f32, tag="tp")
            nc.tensor.transpose(atp, attn_sb, ident)
            attnT = misc_pool.tile([P, Tq], f32)
            nc.scalar.copy(attnT, atp)
            ctx_ps = acc_ps_pool.tile([P, D], f32, tag="ctx")
            nc.tensor.matmul(ctx_ps, lhsT=attnT, rhs=v_sb, start=True, stop=True)
            ctx_sb = misc_pool.tile([P, D], f32)
            nc.vector.tensor_copy(ctx_sb, ctx_ps)
            nc.sync.dma_start(ctx_d[:], ctx_sb[:])

    return nc


def get_bass():
    if "nc" not in _CACHE:
        nc = _build_bass()
        # Run the bacc passes (wait splitting, register allocation, act table
        # loads) and freeze before the PJRT lowering serializes the module.
        nc.finalize()
        _CACHE["nc"] = nc
    return _CACHE["nc"]


def make_in_maps(query, value, mask, W1, W2, scale):
    query = np.ascontiguousarray(np.asarray(query, dtype=np.float32))
    value = np.ascontiguousarray(np.asarray(value, dtype=np.float32))
    mask = np.asarray(mask)
    W1 = np.ascontiguousarray(np.asarray(W1, dtype=np.float32))
    W2 = np.ascontiguousarray(np.asarray(W2, dtype=np.float32))
    scale = np.asarray(scale, dtype=np.float32)
    # scale_pc[p, c] = scale[c*128 + p]
    scale_pc = np.ascontiguousarray(scale.reshape(UC, P).T)
    in_maps = []
    for b in range(B):
        in_maps.append({
            "q": np.ascontiguousarray(query[b]),
            "v": np.ascontiguousarray(value[b]),
            "w1": W1,
            "w2": W2,
            "scale_pc": scale_pc,
            "mask_p": np.ascontiguousarray(
                mask[b].astype(np.uint8).reshape(P, 1)),
        })
    return in_maps


def run(query, value, mask, W1, W2, scale, trace=False):
    from concourse.bass_utils import run_bass_kernel_spmd

    nc = get_bass()
    in_maps = make_in_maps(query, value, mask, W1, W2, scale)
    res = run_bass_kernel_spmd(nc, in_maps, list(range(NCORES)), trace=trace)
    ctx = np.stack([res.results[b]["ctx"] for b in range(B)])
    attn = np.stack([res.results[b]["attn"] for b in range(B)])
    return (ctx, attn), res


def kernel(**inputs):
    (ctx, attn), _ = run(
        inputs["query"], inputs["value"], inputs["mask"],
        inputs["W1"], inputs["W2"], inputs["scale"],
    )
    return ctx, attn


# revision 9
# speedup vs baseline: 3.0897x; 1.1829x over previous
"""Bahdanau (additive) attention kernel for Trainium2, SPMD over 8 NeuronCores.

Problem shapes (hardcoded): B=8, Tq=Tv=128, D=512, U=1024, fp32.
Sharding: data-parallel over batch -- one batch element per core; W1/W2/scale
replicated.

Per-core algorithm:
  wq = q @ W1            [Tq, U]   (kept transposed: [u_part, t])
  wk = v @ W2            [Tv, U]   (kept transposed: [u_part, s])
  scores[t, s] = sum_u scale[u] * tanh(wq[t, u] + wk[s, u])
     - broadcast-add on DVE (tensor_scalar, per-partition scalar = wq column)
     - tanh on ACT engine (big fused tiles)
     - reduction over u via PE matmuls (lhsT = tanh tile [u, s], rhs = scale
       column [u, 1]) accumulating into a PSUM tile laid out [s, t]
  softmax over s (key axis) with max-subtraction, on [t, s] layout
  context = attn @ v via one PE matmul (lhsT = attn^T [s, t], rhs = v [s, d])
"""

import numpy as np

B, Tq, Tv, D, U = 8, 128, 128, 512, 1024
P = 128
UC = U // P  # 8 u-chunks
DC = D // P  # 4 d-chunks
G_T = 4      # query rows per tanh group
NCORES = 8

_CACHE = {}


def _build_bass():
    import concourse.bass as bass
    import concourse.mybir as mybir
    from concourse import bacc
    import concourse.tile as tile
    from concourse.masks import make_identity

    f32 = mybir.dt.float32
    bf16 = mybir.dt.bfloat16
    u8 = mybir.dt.uint8
    AF = mybir.ActivationFunctionType

    nc = bacc.Bacc()

    q_d = nc.declare_dram_parameter("q", [Tq, D], f32, isOutput=False)
    v_d = nc.declare_dram_parameter("v", [Tv, D], f32, isOutput=False)
    w1_d = nc.declare_dram_parameter("w1", [D, U], f32, isOutput=False)
    w2_d = nc.declare_dram_parameter("w2", [D, U], f32, isOutput=False)
    sc_d = nc.declare_dram_parameter("scale_pc", [P, UC], f32, isOutput=False)
    mk_d = nc.declare_dram_parameter("mask_p", [P, 1], u8, isOutput=False)
    ctx_d = nc.declare_dram_parameter("ctx", [Tq, D], f32, isOutput=True)
    attn_d = nc.declare_dram_parameter("attn", [Tq, Tv], f32, isOutput=True)

    with tile.TileContext(nc) as tc:
        with (
            tc.tile_pool(name="const", bufs=1) as const_pool,
            tc.tile_pool(name="w", bufs=1) as w_pool,
            tc.tile_pool(name="sb", bufs=1) as sb_pool,
            tc.tile_pool(name="stage", bufs=3) as stage_pool,
            tc.tile_pool(name="mt", bufs=3) as m_pool,
            tc.tile_pool(name="misc", bufs=2) as misc_pool,
            tc.tile_pool(name="ps", bufs=2, space="PSUM") as ps_pool,
            tc.tile_pool(name="accps", bufs=1, space="PSUM") as acc_ps_pool,
        ):
            ident = const_pool.tile([P, P], f32)
            make_identity(nc, ident)

            # Warm the ACT table set (exp_and_others contains tanh+exp) ASAP
            # so the ~2.7us table load overlaps the input DMAs.
            dummy = const_pool.tile([P, 1], f32)
            nc.vector.memset(dummy, 0.0)
            nc.scalar.activation(dummy, dummy, AF.Tanh)

            scale_sb = const_pool.tile([P, UC], f32)
            nc.sync.dma_start(scale_sb[:], sc_d[:])
            scale_bf = const_pool.tile([P, UC], bf16)
            nc.vector.tensor_copy(scale_bf[:], scale_sb[:])
            mask_u8 = const_pool.tile([P, 1], u8)
            nc.sync.dma_start(mask_u8[:], mk_d[:])
            # mask_neg[s] = (mask[s] - 1) * 1e9  -> 0 where valid, -1e9 where masked
            mask_neg = const_pool.tile([P, 1], f32)
            nc.vector.tensor_scalar(
                out=mask_neg, in0=mask_u8, scalar1=1.0, scalar2=1e9,
                op0=mybir.AluOpType.subtract, op1=mybir.AluOpType.mult,
            )

            q_sb = sb_pool.tile([P, D], f32)
            nc.sync.dma_start(q_sb[:], q_d[:])
            v_sb = sb_pool.tile([P, D], f32)
            nc.sync.dma_start(v_sb[:], v_d[:])
            w1_sb = w_pool.tile([P, DC, U], f32)
            w2_sb = w_pool.tile([P, DC, U], f32)
            w1_r = w1_d.rearrange("(dc p) u -> p dc u", p=P)
            w2_r = w2_d.rearrange("(dc p) u -> p dc u", p=P)
            H = U // 2
            nc.sync.dma_start(w2_sb[:, :, :H], w2_r[:, :, :H])
            nc.sync.dma_start(w1_sb[:, :, :H], w1_r[:, :, :H])
            nc.sync.dma_start(w2_sb[:, :, H:], w2_r[:, :, H:])
            nc.sync.dma_start(w1_sb[:, :, H:], w1_r[:, :, H:])

            # Transpose q, v to [d_part, t/s] chunks for the projection matmuls.
            qT = sb_pool.tile([P, DC, P], f32)
            vT = sb_pool.tile([P, DC, P], f32)
            for src, dst in ((q_sb, qT), (v_sb, vT)):
                for dc in range(DC):
                    tps = ps_pool.tile([P, P], f32, tag="tp")
                    nc.tensor.transpose(tps, src[:, dc * P:(dc + 1) * P], ident)
                    nc.scalar.copy(dst[:, dc, :], tps)

            # Projections: wq[p, c, t] = sum_d W1[d, c*128+p] * q[t, d]
            wq = sb_pool.tile([P, UC, P], f32)
            wk = sb_pool.tile([P, UC, P], bf16)
            for c in range(UC):
                for wsrc, xT, dst in ((w2_sb, vT, wk), (w1_sb, qT, wq)):
                    pps = ps_pool.tile([P, P], f32, tag="proj")
                    for dc in range(DC):
                        nc.tensor.matmul(
                            pps,
                            lhsT=wsrc[:, dc, c * P:(c + 1) * P],
                            rhs=xT[:, dc, :],
                            start=(dc == 0),
                            stop=(dc == DC - 1),
                        )
                    nc.scalar.copy(dst[:, c, :], pps)

            # Main loop: scores_ps[s, t] accumulated over u-chunks.
            scores_ps = acc_ps_pool.tile([P, Tq], f32, tag="scores")
            for g in range(Tq // G_T):
                stage = stage_pool.tile([P, G_T * UC * P], bf16, tag="stage")
                mt = m_pool.tile([P, G_T * UC * P], bf16, tag="mt")
                for tl in range(G_T):
                    t = g * G_T + tl
                    for c in range(UC):
                        off = (tl * UC + c) * P
                        nc.vector.tensor_scalar_add(
                            out=stage[:, off:off + P],
                            in0=wk[:, c, :],
                            scalar1=wq[:, c, t:t + 1],
                        )
                nc.scalar.activation(mt, stage, AF.Tanh)
                for tl in range(G_T):
                    t = g * G_T + tl
                    for c in range(UC):
                        off = (tl * UC + c) * P
                        nc.tensor.matmul(
                            scores_ps[:, t:t + 1],
                            lhsT=mt[:, off:off + P],
                            rhs=scale_bf[:, c:c + 1],
                            start=(c == 0),
                            stop=(c == UC - 1),
                        )

            # Mask + evacuate psum (scores laid out [s, t]).
            scores_sb = misc_pool.tile([P, Tq], f32)
            nc.vector.tensor_scalar_add(scores_sb, scores_ps, mask_neg)

            # Transpose to [t, s] for free-axis softmax.
            tsp = ps_pool.tile([P, P], f32, tag="tp")
            nc.tensor.transpose(tsp, scores_sb, ident)
            neg_mx = misc_pool.tile([P, 1], f32)
            nc.vector.reduce_max(neg_mx, tsp, axis=mybir.AxisListType.X, negate=True)
            p_sb = misc_pool.tile([P, Tv], f32)
            sum_sb = misc_pool.tile([P, 1], f32)
            nc.scalar.activation(p_sb, tsp, AF.Exp, bias=neg_mx, accum_out=sum_sb)
            rsum = misc_pool.tile([P, 1], f32)
            nc.vector.reciprocal(rsum, sum_sb)
            attn_sb = misc_pool.tile([P, Tv], f32)
            nc.vector.tensor_scalar_mul(attn_sb, p_sb, rsum)
            nc.sync.dma_start(attn_d[:], attn_sb[:])

            # context = attn @ v : lhsT = attn^T [s, t], rhs = v [s, d]
            atp = ps_pool.tile([P, P], f32, tag="tp")
            nc.tensor.transpose(atp, attn_sb, ident)
            attnT = misc_pool.tile([P, Tq], f32)
            nc.scalar.copy(attnT, atp)
            ctx_ps = acc_ps_pool.tile([P, D], f32, tag="ctx")
            nc.tensor.matmul(ctx_ps, lhsT=attnT, rhs=v_sb, start=True, stop=True)
            ctx_sb = misc_pool.tile([P, D], f32)
            nc.vector.tensor_copy(ctx_sb, ctx_ps)
            nc.sync.dma_start(ctx_d[:], ctx_sb[:])

    return nc


def get_bass():
    if "nc" not in _CACHE:
        nc = _build_bass()
        # Run the bacc passes (wait splitting, register allocation, act table
        # loads) and freeze before the PJRT lowering serializes the module.
        nc.finalize()
        _CACHE["nc"] = nc
    return _CACHE["nc"]


def make_in_maps(query, value, mask, W1, W2, scale):
    query = np.ascontiguousarray(np.asarray(query, dtype=np.float32))
    value = np.ascontiguousarray(np.asarray(value, dtype=np.float32))
    mask = np.asarray(mask)
    W1 = np.ascontiguousarray(np.asarray(W1, dtype=np.float32))
    W2 = np.ascontiguousarray(np.asarray(W2, dtype=np.float32))
    scale = np.asarray(scale, dtype=np.float32)
    # scale_pc[p, c] = scale[c*128 + p]
    scale_pc = np.ascontiguousarray(scale.reshape(UC, P).T)
    in_maps = []
    for b in range(B):
        in_maps.append({
            "q": np.ascontiguousarray(query[b]),
            "v": np.ascontiguousarray(value[b]),
            "w1": W1,
            "w2": W2,
            "scale_pc": scale_pc,
            "mask_p": np.ascontiguousarray(
                mask[b].astype(np.uint8).reshape(P, 1)),
        })
    return in_maps


def run(query, value, mask, W1, W2, scale, trace=False):
    from concourse.bass_utils import run_bass_kernel_spmd

    nc = get_bass()
    in_maps = make_in_maps(query, value, mask, W1, W2, scale)
    res = run_bass_kernel_spmd(nc, in_maps, list(range(NCORES)), trace=trace)
    ctx = np.stack([res.results[b]["ctx"] for b in range(B)])
    attn = np.stack([res.results[b]["attn"] for b in range(B)])
    return (ctx, attn), res


def kernel(**inputs):
    (ctx, attn), _ = run(
        inputs["query"], inputs["value"], inputs["mask"],
        inputs["W1"], inputs["W2"], inputs["scale"],
    )
    return ctx, attn


# revision 11
# speedup vs baseline: 3.3255x; 1.0763x over previous
"""Bahdanau (additive) attention kernel for Trainium2, SPMD over 8 NeuronCores.

Problem shapes (hardcoded): B=8, Tq=Tv=128, D=512, U=1024, fp32.
Sharding: data-parallel over batch -- one batch element per core; W1/W2/scale
replicated.

Per-core algorithm:
  wq = q @ W1            [Tq, U]   (kept transposed: [u_part, t])
  wk = v @ W2            [Tv, U]   (kept transposed: [u_part, s])
  scores[t, s] = sum_u scale[u] * tanh(wq[t, u] + wk[s, u])
     - broadcast-add on DVE (tensor_scalar, per-partition scalar = wq column)
     - tanh on ACT engine (big fused tiles)
     - reduction over u via PE matmuls (lhsT = tanh tile [u, s], rhs = scale
       column [u, 1]) accumulating into a PSUM tile laid out [s, t]
  softmax over s (key axis) with max-subtraction, on [t, s] layout
  context = attn @ v via one PE matmul (lhsT = attn^T [s, t], rhs = v [s, d])
"""

import numpy as np

B, Tq, Tv, D, U = 8, 128, 128, 512, 1024
P = 128
UC = U // P  # 8 u-chunks
DC = D // P  # 4 d-chunks
G_T = 4      # query rows per tanh group
NCORES = 8

_CACHE = {}


def _build_bass():
    import concourse.bass as bass
    import concourse.mybir as mybir
    from concourse import bacc
    import concourse.tile as tile
    from concourse.masks import make_identity

    f32 = mybir.dt.float32
    bf16 = mybir.dt.bfloat16
    u8 = mybir.dt.uint8
    AF = mybir.ActivationFunctionType

    nc = bacc.Bacc()

    q_d = nc.declare_dram_parameter("q", [Tq, D], f32, isOutput=False)
    v_d = nc.declare_dram_parameter("v", [Tv, D], f32, isOutput=False)
    w1_d = nc.declare_dram_parameter("w1", [D, U], f32, isOutput=False)
    w2_d = nc.declare_dram_parameter("w2", [D, U], f32, isOutput=False)
    sc_d = nc.declare_dram_parameter("scale_pc", [P, UC], f32, isOutput=False)
    mk_d = nc.declare_dram_parameter("mask_p", [P, 1], u8, isOutput=False)
    ctx_d = nc.declare_dram_parameter("ctx", [Tq, D], f32, isOutput=True)
    attn_d = nc.declare_dram_parameter("attn", [Tq, Tv], f32, isOutput=True)

    with tile.TileContext(nc) as tc:
        with (
            tc.tile_pool(name="const", bufs=1) as const_pool,
            tc.tile_pool(name="w", bufs=1) as w_pool,
            tc.tile_pool(name="sb", bufs=1) as sb_pool,
            tc.tile_pool(name="stage", bufs=3) as stage_pool,
            tc.tile_pool(name="mt", bufs=3) as m_pool,
            tc.tile_pool(name="misc", bufs=2) as misc_pool,
            tc.tile_pool(name="ps", bufs=2, space="PSUM") as ps_pool,
            tc.tile_pool(name="accps", bufs=1, space="PSUM") as acc_ps_pool,
            tc.tile_pool(name="stgps", bufs=2, space="PSUM") as stg_ps_pool,
        ):
            ident = const_pool.tile([P, P], f32)
            make_identity(nc, ident)
            ident_bf = const_pool.tile([P, P], bf16)
            make_identity(nc, ident_bf)

            # Warm the ACT table set (exp_and_others contains tanh+exp) ASAP
            # so the ~2.7us table load overlaps the input DMAs.
            dummy = const_pool.tile([P, 1], f32)
            nc.vector.memset(dummy, 0.0)
            nc.scalar.activation(dummy, dummy, AF.Tanh)

            scale_sb = const_pool.tile([P, UC], f32)
            nc.sync.dma_start(scale_sb[:], sc_d[:])
            scale_bf = const_pool.tile([P, UC], bf16)
            nc.vector.tensor_copy(scale_bf[:], scale_sb[:])
            mask_u8 = const_pool.tile([P, 1], u8)
            nc.sync.dma_start(mask_u8[:], mk_d[:])
            # mask_neg[s] = (mask[s] - 1) * 1e9  -> 0 where valid, -1e9 where masked
            mask_neg = const_pool.tile([P, 1], f32)
            nc.vector.tensor_scalar(
                out=mask_neg, in0=mask_u8, scalar1=1.0, scalar2=1e9,
                op0=mybir.AluOpType.subtract, op1=mybir.AluOpType.mult,
            )

            q_sb = sb_pool.tile([P, D], f32)
            nc.sync.dma_start(q_sb[:], q_d[:])
            v_sb = sb_pool.tile([P, D], f32)
            nc.sync.dma_start(v_sb[:], v_d[:])
            w1_sb = w_pool.tile([P, DC, U], f32)
            w2_sb = w_pool.tile([P, DC, U], f32)
            w1_r = w1_d.rearrange("(dc p) u -> p dc u", p=P)
            w2_r = w2_d.rearrange("(dc p) u -> p dc u", p=P)
            H = U // 2
            nc.sync.dma_start(w2_sb[:, :, :H], w2_r[:, :, :H])
            nc.sync.dma_start(w1_sb[:, :, :H], w1_r[:, :, :H])
            nc.sync.dma_start(w2_sb[:, :, H:], w2_r[:, :, H:])
            nc.sync.dma_start(w1_sb[:, :, H:], w1_r[:, :, H:])

            # Transpose q, v to [d_part, t/s] chunks for the projection matmuls.
            qT = sb_pool.tile([P, DC, P], f32)
            vT = sb_pool.tile([P, DC, P], f32)
            for src, dst in ((q_sb, qT), (v_sb, vT)):
                for dc in range(DC):
                    tps = ps_pool.tile([P, P], f32, tag="tp")
                    nc.tensor.transpose(tps, src[:, dc * P:(dc + 1) * P], ident)
                    nc.scalar.copy(dst[:, dc, :], tps)

            # Projections: wq[p, c, t] = sum_d W1[d, c*128+p] * q[t, d]
            wq = sb_pool.tile([P, UC, P], f32)
            wk = sb_pool.tile([P, UC, P], bf16)
            for c in range(UC):
                for wsrc, xT, dst in ((w2_sb, vT, wk), (w1_sb, qT, wq)):
                    pps = ps_pool.tile([P, P], f32, tag="proj")
                    for dc in range(DC):
                        nc.tensor.matmul(
                            pps,
                            lhsT=wsrc[:, dc, c * P:(c + 1) * P],
                            rhs=xT[:, dc, :],
                            start=(dc == 0),
                            stop=(dc == DC - 1),
                        )
                    nc.scalar.copy(dst[:, c, :], pps)

            # Natural-layout bf16 slices of wq/wk for the PE-offloaded
            # broadcast-add chunks (c in {6, 7}).
            PE_CHUNKS = (6, 7)
            wqn = sb_pool.tile([P, len(PE_CHUNKS), P], bf16)
            wkn = sb_pool.tile([P, len(PE_CHUNKS), P], bf16)
            for j, c in enumerate(PE_CHUNKS):
                tq = ps_pool.tile([P, P], f32, tag="tp")
                nc.tensor.transpose(tq, wq[:, c, :], ident)
                nc.scalar.copy(wqn[:, j, :], tq)
                tk = ps_pool.tile([P, P], bf16, tag="tp")
                nc.tensor.transpose(tk, wk[:, c, :], ident_bf)
                nc.scalar.copy(wkn[:, j, :], tk)

            # Main loop: scores_ps[s, t] accumulated over u-chunks.
            scores_ps = acc_ps_pool.tile([P, Tq], f32, tag="scores")
            NDV = UC - len(PE_CHUNKS)  # chunks handled by DVE adds
            for g in range(Tq // G_T):
                t0 = g * G_T
                stage = stage_pool.tile([P, G_T, NDV * P], bf16, tag="stage")
                mt = m_pool.tile([P, G_T, UC, P], bf16, tag="mt")
                for tl in range(G_T):
                    t = t0 + tl
                    for c in range(NDV):
                        nc.vector.tensor_scalar_add(
                            out=stage[:, tl, c * P:(c + 1) * P],
                            in0=wk[:, c, :],
                            scalar1=wq[:, c, t:t + 1],
                        )
                nc.scalar.activation(mt[:, :, :NDV, :], stage, AF.Tanh)
                for j, c in enumerate(PE_CHUNKS):
                    stg = stg_ps_pool.tile([P, G_T, P], f32, tag="stg")
                    nc.tensor.matmul(
                        stg[:],
                        lhsT=wqn[:, j, :],
                        rhs=ident_bf[:, t0:t0 + G_T, None].to_broadcast((P, G_T, P)),
                        start=True, stop=False,
                    )
                    nc.tensor.matmul(
                        stg[:],
                        lhsT=wkn[:, j, :],
                        rhs=ident_bf[:, None, :].to_broadcast((P, G_T, P)),
                        start=False, stop=True,
                    )
                    nc.scalar.activation(mt[:, :, c, :], stg, AF.Tanh)
                for tl in range(G_T):
                    t = t0 + tl
                    for c in range(UC):
                        nc.tensor.matmul(
                            scores_ps[:, t:t + 1],
                            lhsT=mt[:, tl, c, :],
                            rhs=scale_bf[:, c:c + 1],
                            start=(c == 0),
                            stop=(c == UC - 1),
                        )

            # Mask + evacuate psum (scores laid out [s, t]).
            scores_sb = misc_pool.tile([P, Tq], f32)
            nc.vector.tensor_scalar_add(scores_sb, scores_ps, mask_neg)

            # Transpose to [t, s] for free-axis softmax.
            tsp = ps_pool.tile([P, P], f32, tag="tp")
            nc.tensor.transpose(tsp, scores_sb, ident)
            neg_mx = misc_pool.tile([P, 1], f32)
            nc.vector.reduce_max(neg_mx, tsp, axis=mybir.AxisListType.X, negate=True)
            p_sb = misc_pool.tile([P, Tv], f32)
            sum_sb = misc_pool.tile([P, 1], f32)
            nc.scalar.activation(p_sb, tsp, AF.Exp, bias=neg_mx, accum_out=sum_sb)
            rsum = misc_pool.tile([P, 1], f32)
            nc.vector.reciprocal(rsum, sum_sb)
            attn_sb = misc_pool.tile([P, Tv], f32)
            nc.vector.tensor_scalar_mul(attn_sb, p_sb, rsum)
            nc.sync.dma_start(attn_d[:], attn_sb[:])

            # context = attn @ v : lhsT = attn^T [s, t], rhs = v [s, d]
            atp = ps_pool.tile([P, P], f32, tag="tp")
            nc.tensor.transpose(atp, attn_sb, ident)
            attnT = misc_pool.tile([P, Tq], f32)
            nc.scalar.copy(attnT, atp)
            ctx_ps = acc_ps_pool.tile([P, D], f32, tag="ctx")
            nc.tensor.matmul(ctx_ps, lhsT=attnT, rhs=v_sb, start=True, stop=True)
            ctx_sb = misc_pool.tile([P, D], f32)
            nc.vector.tensor_copy(ctx_sb, ctx_ps)
            nc.sync.dma_start(ctx_d[:], ctx_sb[:])

    return nc


def get_bass():
    if "nc" not in _CACHE:
        nc = _build_bass()
        # Run the bacc passes (wait splitting, register allocation, act table
        # loads) and freeze before the PJRT lowering serializes the module.
        nc.finalize()
        _CACHE["nc"] = nc
    return _CACHE["nc"]


def make_in_maps(query, value, mask, W1, W2, scale):
    query = np.ascontiguousarray(np.asarray(query, dtype=np.float32))
    value = np.ascontiguousarray(np.asarray(value, dtype=np.float32))
    mask = np.asarray(mask)
    W1 = np.ascontiguousarray(np.asarray(W1, dtype=np.float32))
    W2 = np.ascontiguousarray(np.asarray(W2, dtype=np.float32))
    scale = np.asarray(scale, dtype=np.float32)
    # scale_pc[p, c] = scale[c*128 + p]
    scale_pc = np.ascontiguousarray(scale.reshape(UC, P).T)
    in_maps = []
    for b in range(B):
        in_maps.append({
            "q": np.ascontiguousarray(query[b]),
            "v": np.ascontiguousarray(value[b]),
            "w1": W1,
            "w2": W2,
            "scale_pc": scale_pc,
            "mask_p": np.ascontiguousarray(
                mask[b].astype(np.uint8).reshape(P, 1)),
        })
    return in_maps


def run(query, value, mask, W1, W2, scale, trace=False):
    from concourse.bass_utils import run_bass_kernel_spmd

    nc = get_bass()
    in_maps = make_in_maps(query, value, mask, W1, W2, scale)
    res = run_bass_kernel_spmd(nc, in_maps, list(range(NCORES)), trace=trace)
    ctx = np.stack([res.results[b]["ctx"] for b in range(B)])
    attn = np.stack([res.results[b]["attn"] for b in range(B)])
    return (ctx, attn), res


def kernel(**inputs):
    (ctx, attn), _ = run(
        inputs["query"], inputs["value"], inputs["mask"],
        inputs["W1"], inputs["W2"], inputs["scale"],
    )
    return ctx, attn


# revision 13
# speedup vs baseline: 3.6184x; 1.0881x over previous
"""Bahdanau (additive) attention kernel for Trainium2, SPMD over 8 NeuronCores.

Problem shapes (hardcoded): B=8, Tq=Tv=128, D=512, U=1024, fp32.
Sharding: data-parallel over batch -- one batch element per core; W1/W2/scale
replicated.

Per-core algorithm:
  wq = q @ W1            [Tq, U]   (kept transposed: [u_part, t])
  wk = v @ W2            [Tv, U]   (kept transposed: [u_part, s])
  scores[t, s] = sum_u scale[u] * tanh(wq[t, u] + wk[s, u])
     - broadcast-add on DVE (tensor_scalar, per-partition scalar = wq column)
     - tanh on ACT engine (big fused tiles)
     - reduction over u via PE matmuls (lhsT = tanh tile [u, s], rhs = scale
       column [u, 1]) accumulating into a PSUM tile laid out [s, t]
  softmax over s (key axis) with max-subtraction, on [t, s] layout
  context = attn @ v via one PE matmul (lhsT = attn^T [s, t], rhs = v [s, d])
"""

import numpy as np

B, Tq, Tv, D, U = 8, 128, 128, 512, 1024
P = 128
UC = U // P  # 8 u-chunks
DC = D // P  # 4 d-chunks
G_T = 4      # query rows per tanh group
NCORES = 8

_CACHE = {}


def _build_bass():
    import concourse.bass as bass
    import concourse.mybir as mybir
    from concourse import bacc
    import concourse.tile as tile
    from concourse.masks import make_identity

    f32 = mybir.dt.float32
    bf16 = mybir.dt.bfloat16
    u8 = mybir.dt.uint8
    AF = mybir.ActivationFunctionType

    nc = bacc.Bacc()

    q_d = nc.declare_dram_parameter("q", [Tq, D], f32, isOutput=False)
    v_d = nc.declare_dram_parameter("v", [Tv, D], f32, isOutput=False)
    w1_d = nc.declare_dram_parameter("w1", [D, U], f32, isOutput=False)
    w2_d = nc.declare_dram_parameter("w2", [D, U], f32, isOutput=False)
    sc_d = nc.declare_dram_parameter("scale_pc", [P, UC], f32, isOutput=False)
    mk_d = nc.declare_dram_parameter("mask_p", [P, 1], u8, isOutput=False)
    ctx_d = nc.declare_dram_parameter("ctx", [Tq, D], f32, isOutput=True)
    attn_d = nc.declare_dram_parameter("attn", [Tq, Tv], f32, isOutput=True)

    with tile.TileContext(nc) as tc:
        with (
            tc.tile_pool(name="const", bufs=1) as const_pool,
            tc.tile_pool(name="w", bufs=1) as w_pool,
            tc.tile_pool(name="sb", bufs=1) as sb_pool,
            tc.tile_pool(name="stage", bufs=4) as stage_pool,
            tc.tile_pool(name="mt", bufs=4) as m_pool,
            tc.tile_pool(name="misc", bufs=2) as misc_pool,
            tc.tile_pool(name="ps", bufs=2, space="PSUM") as ps_pool,
            tc.tile_pool(name="accps", bufs=1, space="PSUM") as acc_ps_pool,
            tc.tile_pool(name="stgps", bufs=2, space="PSUM") as stg_ps_pool,
        ):
            ident = const_pool.tile([P, P], f32)
            make_identity(nc, ident)
            ident_bf = const_pool.tile([P, P], bf16)
            make_identity(nc, ident_bf)
            oh_s = const_pool.tile([P, 4, P], bf16)
            for tl in range(4):
                nc.vector.tensor_copy(oh_s[:, tl, :], ident_bf[:])

            # Warm the ACT table set (exp_and_others contains tanh+exp) ASAP
            # so the ~2.7us table load overlaps the input DMAs.
            dummy = const_pool.tile([P, 1], f32)
            nc.vector.memset(dummy, 0.0)
            nc.scalar.activation(dummy, dummy, AF.Tanh)

            scale_sb = const_pool.tile([P, UC], f32)
            nc.sync.dma_start(scale_sb[:], sc_d[:])
            scale_bf = const_pool.tile([P, UC], bf16)
            nc.vector.tensor_copy(scale_bf[:], scale_sb[:])
            mask_u8 = const_pool.tile([P, 1], u8)
            nc.sync.dma_start(mask_u8[:], mk_d[:])
            # mask_neg[s] = (mask[s] - 1) * 1e9  -> 0 where valid, -1e9 where masked
            mask_neg = const_pool.tile([P, 1], f32)
            nc.vector.tensor_scalar(
                out=mask_neg, in0=mask_u8, scalar1=1.0, scalar2=1e9,
                op0=mybir.AluOpType.subtract, op1=mybir.AluOpType.mult,
            )

            q_sb = sb_pool.tile([P, D], f32)
            nc.sync.dma_start(q_sb[:], q_d[:])
            v_sb = sb_pool.tile([P, D], f32)
            nc.sync.dma_start(v_sb[:], v_d[:])
            w1_sb = w_pool.tile([P, DC, U], f32)
            w2_sb = w_pool.tile([P, DC, U], f32)
            w1_r = w1_d.rearrange("(dc p) u -> p dc u", p=P)
            w2_r = w2_d.rearrange("(dc p) u -> p dc u", p=P)
            H = U // 2
            nc.sync.dma_start(w2_sb[:, :, :H], w2_r[:, :, :H])
            nc.sync.dma_start(w1_sb[:, :, :H], w1_r[:, :, :H])
            nc.sync.dma_start(w2_sb[:, :, H:], w2_r[:, :, H:])
            nc.sync.dma_start(w1_sb[:, :, H:], w1_r[:, :, H:])

            # Transpose q, v to [d_part, t/s] chunks for the projection matmuls.
            qT = sb_pool.tile([P, DC, P], f32)
            vT = sb_pool.tile([P, DC, P], f32)
            for src, dst in ((q_sb, qT), (v_sb, vT)):
                for dc in range(DC):
                    tps = ps_pool.tile([P, P], f32, tag="tp")
                    nc.tensor.transpose(tps, src[:, dc * P:(dc + 1) * P], ident)
                    nc.vector.tensor_copy(dst[:, dc, :], tps)

            # Projections: wq[p, c, t] = sum_d W1[d, c*128+p] * q[t, d]
            wq = sb_pool.tile([P, UC, P], f32)
            wk = sb_pool.tile([P, UC, P], bf16)
            for c in range(UC):
                for wsrc, xT, dst in ((w2_sb, vT, wk), (w1_sb, qT, wq)):
                    pps = ps_pool.tile([P, P], f32, tag="proj")
                    for dc in range(DC):
                        nc.tensor.matmul(
                            pps,
                            lhsT=wsrc[:, dc, c * P:(c + 1) * P],
                            rhs=xT[:, dc, :],
                            start=(dc == 0),
                            stop=(dc == DC - 1),
                        )
                    nc.vector.tensor_copy(dst[:, c, :], pps)

            # Natural-layout bf16 slices of wq/wk for the PE-offloaded
            # broadcast-add chunks (c in {6, 7}).
            PE_CHUNKS = (6, 7)
            wqn = sb_pool.tile([P, len(PE_CHUNKS), P], bf16)
            wkn = sb_pool.tile([P, len(PE_CHUNKS), P], bf16)
            for j, c in enumerate(PE_CHUNKS):
                tq = ps_pool.tile([P, P], f32, tag="tp")
                nc.tensor.transpose(tq, wq[:, c, :], ident)
                nc.scalar.copy(wqn[:, j, :], tq)
                tk = ps_pool.tile([P, P], bf16, tag="tp")
                nc.tensor.transpose(tk, wk[:, c, :], ident_bf)
                nc.scalar.copy(wkn[:, j, :], tk)

            # Main loop: scores_ps[s, t] accumulated over u-chunks.
            scores_ps = acc_ps_pool.tile([P, Tq], f32, tag="scores")
            NDV = UC - len(PE_CHUNKS)  # chunks handled by DVE adds
            for g in range(Tq // G_T):
                t0 = g * G_T
                stage = stage_pool.tile([P, G_T, NDV * P], bf16, tag="stage")
                mt = m_pool.tile([P, G_T, UC, P], bf16, tag="mt")
                for tl in range(G_T):
                    t = t0 + tl
                    for c in range(NDV):
                        nc.vector.tensor_scalar_add(
                            out=stage[:, tl, c * P:(c + 1) * P],
                            in0=wk[:, c, :],
                            scalar1=wq[:, c, t:t + 1],
                        )
                nc.scalar.activation(mt[:, :, :NDV, :], stage, AF.Tanh)
                for j, c in enumerate(PE_CHUNKS):
                    stg = stg_ps_pool.tile([P, G_T, P], f32, tag="stg")
                    nc.tensor.matmul(
                        stg[:],
                        lhsT=wqn[:, j, :],
                        rhs=ident_bf[:, t0:t0 + G_T, None].to_broadcast((P, G_T, P)),
                        start=True, stop=False,
                    )
                    nc.tensor.matmul(
                        stg[:],
                        lhsT=wkn[:, j, :],
                        rhs=oh_s[:],
                        start=False, stop=True,
                    )
                    nc.scalar.activation(mt[:, :, c, :], stg, AF.Tanh)
                for tl in range(G_T):
                    t = t0 + tl
                    for c in range(UC):
                        nc.tensor.matmul(
                            scores_ps[:, t:t + 1],
                            lhsT=mt[:, tl, c, :],
                            rhs=scale_bf[:, c:c + 1],
                            start=(c == 0),
                            stop=(c == UC - 1),
                        )

            # Softmax + context in two t-halves so the first half overlaps
            # the tail of the main loop. scores_ps is laid out [s, t].
            HT = Tq // 2
            for h in range(2):
                hs = h * HT
                sc_h = misc_pool.tile([P, HT], f32, tag="sc_h")
                nc.vector.tensor_scalar_add(
                    sc_h, scores_ps[:, hs:hs + HT], mask_neg)
                tsp = ps_pool.tile([HT, P], f32, tag="tp")
                nc.tensor.transpose(tsp, sc_h, ident)
                neg_mx = misc_pool.tile([HT, 1], f32, tag="mx")
                nc.vector.reduce_max(
                    neg_mx, tsp, axis=mybir.AxisListType.X, negate=True)
                p_sb = misc_pool.tile([HT, Tv], f32, tag="p_h")
                sum_sb = misc_pool.tile([HT, 1], f32, tag="sum_h")
                nc.scalar.activation(p_sb, tsp, AF.Exp, bias=neg_mx,
                                     accum_out=sum_sb)
                rsum = misc_pool.tile([HT, 1], f32, tag="rs_h")
                nc.vector.reciprocal(rsum, sum_sb)
                attn_sb = misc_pool.tile([HT, Tv], f32, tag="attn_h")
                nc.vector.tensor_scalar_mul(attn_sb, p_sb, rsum)
                nc.sync.dma_start(attn_d[hs:hs + HT, :], attn_sb[:])
                atp = ps_pool.tile([P, HT], f32, tag="tp")
                nc.tensor.transpose(atp, attn_sb, ident[:HT, :HT])
                attnT = misc_pool.tile([P, HT], f32, tag="attnT_h")
                nc.scalar.copy(attnT, atp)
                ctx_ps = acc_ps_pool.tile([HT, D], f32, tag="ctx")
                nc.tensor.matmul(ctx_ps, lhsT=attnT, rhs=v_sb,
                                 start=True, stop=True)
                ctx_sb = misc_pool.tile([HT, D], f32, tag="ctx_h")
                nc.vector.tensor_copy(ctx_sb, ctx_ps)
                nc.sync.dma_start(ctx_d[hs:hs + HT, :], ctx_sb[:])

    return nc


def get_bass():
    if "nc" not in _CACHE:
        nc = _build_bass()
        # Run the bacc passes (wait splitting, register allocation, act table
        # loads) and freeze before the PJRT lowering serializes the module.
        nc.finalize()
        _CACHE["nc"] = nc
    return _CACHE["nc"]


def make_in_maps(query, value, mask, W1, W2, scale):
    query = np.ascontiguousarray(np.asarray(query, dtype=np.float32))
    value = np.ascontiguousarray(np.asarray(value, dtype=np.float32))
    mask = np.asarray(mask)
    W1 = np.ascontiguousarray(np.asarray(W1, dtype=np.float32))
    W2 = np.ascontiguousarray(np.asarray(W2, dtype=np.float32))
    scale = np.asarray(scale, dtype=np.float32)
    # scale_pc[p, c] = scale[c*128 + p]
    scale_pc = np.ascontiguousarray(scale.reshape(UC, P).T)
    in_maps = []
    for b in range(B):
        in_maps.append({
            "q": np.ascontiguousarray(query[b]),
            "v": np.ascontiguousarray(value[b]),
            "w1": W1,
            "w2": W2,
            "scale_pc": scale_pc,
            "mask_p": np.ascontiguousarray(
                mask[b].astype(np.uint8).reshape(P, 1)),
        })
    return in_maps


def run(query, value, mask, W1, W2, scale, trace=False):
    from concourse.bass_utils import run_bass_kernel_spmd

    nc = get_bass()
    in_maps = make_in_maps(query, value, mask, W1, W2, scale)
    res = run_bass_kernel_spmd(nc, in_maps, list(range(NCORES)), trace=trace)
    ctx = np.stack([res.results[b]["ctx"] for b in range(B)])
    attn = np.stack([res.results[b]["attn"] for b in range(B)])
    return (ctx, attn), res


def kernel(**inputs):
    (ctx, attn), _ = run(
        inputs["query"], inputs["value"], inputs["mask"],
        inputs["W1"], inputs["W2"], inputs["scale"],
    )
    return ctx, attn


# revision 16
# speedup vs baseline: 3.6218x; 1.0009x over previous
"""Bahdanau (additive) attention kernel for Trainium2, SPMD over 8 NeuronCores.

Problem shapes (hardcoded): B=8, Tq=Tv=128, D=512, U=1024, fp32.
Sharding: data-parallel over batch -- one batch element per core; W1/W2/scale
replicated.

Per-core algorithm:
  wq = q @ W1            [Tq, U]   (kept transposed: [u_part, t])
  wk = v @ W2            [Tv, U]   (kept transposed: [u_part, s])
  scores[t, s] = sum_u scale[u] * tanh(wq[t, u] + wk[s, u])
     - broadcast-add on DVE (tensor_scalar, per-partition scalar = wq column)
     - tanh on ACT engine (big fused tiles)
     - reduction over u via PE matmuls (lhsT = tanh tile [u, s], rhs = scale
       column [u, 1]) accumulating into a PSUM tile laid out [s, t]
  softmax over s (key axis) with max-subtraction, on [t, s] layout
  context = attn @ v via one PE matmul (lhsT = attn^T [s, t], rhs = v [s, d])
"""

import numpy as np

B, Tq, Tv, D, U = 8, 128, 128, 512, 1024
P = 128
UC = U // P  # 8 u-chunks
DC = D // P  # 4 d-chunks
G_T = 4      # query rows per tanh group
NCORES = 8

_CACHE = {}


def _build_bass():
    import concourse.bass as bass
    import concourse.mybir as mybir
    from concourse import bacc
    import concourse.tile as tile
    from concourse.masks import make_identity

    f32 = mybir.dt.float32
    bf16 = mybir.dt.bfloat16
    u8 = mybir.dt.uint8
    AF = mybir.ActivationFunctionType

    nc = bacc.Bacc()

    q_d = nc.declare_dram_parameter("q", [Tq, D], f32, isOutput=False)
    v_d = nc.declare_dram_parameter("v", [Tv, D], f32, isOutput=False)
    w1_d = nc.declare_dram_parameter("w1", [D, U], f32, isOutput=False)
    w2_d = nc.declare_dram_parameter("w2", [D, U], f32, isOutput=False)
    sc_d = nc.declare_dram_parameter("scale_pc", [P, UC], f32, isOutput=False)
    mk_d = nc.declare_dram_parameter("mask_p", [P, 1], u8, isOutput=False)
    ctx_d = nc.declare_dram_parameter("ctx", [Tq, D], f32, isOutput=True)
    attn_d = nc.declare_dram_parameter("attn", [Tq, Tv], f32, isOutput=True)

    with tile.TileContext(nc) as tc:
        with (
            tc.tile_pool(name="const", bufs=1) as const_pool,
            tc.tile_pool(name="w", bufs=1) as w_pool,
            tc.tile_pool(name="sb", bufs=1) as sb_pool,
            tc.tile_pool(name="stage", bufs=4) as stage_pool,
            tc.tile_pool(name="mt", bufs=4) as m_pool,
            tc.tile_pool(name="misc", bufs=2) as misc_pool,
            tc.tile_pool(name="ps", bufs=2, space="PSUM") as ps_pool,
            tc.tile_pool(name="accps", bufs=1, space="PSUM") as acc_ps_pool,
            tc.tile_pool(name="stgps", bufs=2, space="PSUM") as stg_ps_pool,
        ):
            ident = const_pool.tile([P, P], f32)
            make_identity(nc, ident)
            ident_bf = const_pool.tile([P, P], bf16)
            make_identity(nc, ident_bf)
            oh_s = const_pool.tile([P, 4, P], bf16)
            for tl in range(4):
                nc.vector.tensor_copy(oh_s[:, tl, :], ident_bf[:])

            # Warm the ACT table set (exp_and_others contains tanh+exp) ASAP
            # so the ~2.7us table load overlaps the input DMAs.
            dummy = const_pool.tile([P, 1], f32)
            nc.vector.memset(dummy, 0.0)
            nc.scalar.activation(dummy, dummy, AF.Tanh)

            scale_sb = const_pool.tile([P, UC], f32)
            nc.sync.dma_start(scale_sb[:], sc_d[:])
            scale_bf = const_pool.tile([P, UC], bf16)
            nc.vector.tensor_copy(scale_bf[:], scale_sb[:])
            mask_u8 = const_pool.tile([P, 1], u8)
            nc.sync.dma_start(mask_u8[:], mk_d[:])
            # mask_neg[s] = (mask[s] - 1) * 1e9  -> 0 where valid, -1e9 where masked
            mask_neg = const_pool.tile([P, 1], f32)
            nc.vector.tensor_scalar(
                out=mask_neg, in0=mask_u8, scalar1=1.0, scalar2=1e9,
                op0=mybir.AluOpType.subtract, op1=mybir.AluOpType.mult,
            )

            q_sb = sb_pool.tile([P, D], f32)
            nc.sync.dma_start(q_sb[:], q_d[:])
            v_sb = sb_pool.tile([P, D], f32)
            nc.sync.dma_start(v_sb[:], v_d[:])
            w1_sb = w_pool.tile([P, DC, U], f32)
            w2_sb = w_pool.tile([P, DC, U], f32)
            w1_r = w1_d.rearrange("(dc p) u -> p dc u", p=P)
            w2_r = w2_d.rearrange("(dc p) u -> p dc u", p=P)
            H = U // 2
            nc.sync.dma_start(w2_sb[:, :, :H], w2_r[:, :, :H])
            nc.sync.dma_start(w1_sb[:, :, :H], w1_r[:, :, :H])
            nc.sync.dma_start(w2_sb[:, :, H:], w2_r[:, :, H:])
            nc.sync.dma_start(w1_sb[:, :, H:], w1_r[:, :, H:])

            # Transpose q, v to [d_part, t/s] chunks for the projection matmuls.
            qT = sb_pool.tile([P, DC, P], f32)
            vT = sb_pool.tile([P, DC, P], f32)
            for src, dst in ((q_sb, qT), (v_sb, vT)):
                for dc in range(DC):
                    tps = ps_pool.tile([P, P], f32, tag="tp")
                    nc.tensor.transpose(tps, src[:, dc * P:(dc + 1) * P], ident)
                    nc.vector.tensor_copy(dst[:, dc, :], tps)

            # Projections: wq[p, c, t] = sum_d W1[d, c*128+p] * q[t, d]
            wq = sb_pool.tile([P, UC, P], f32)
            wk = sb_pool.tile([P, UC, P], bf16)
            for c in range(UC):
                for wsrc, xT, dst in ((w2_sb, vT, wk), (w1_sb, qT, wq)):
                    pps = ps_pool.tile([P, P], f32, tag="proj")
                    for dc in range(DC):
                        nc.tensor.matmul(
                            pps,
                            lhsT=wsrc[:, dc, c * P:(c + 1) * P],
                            rhs=xT[:, dc, :],
                            start=(dc == 0),
                            stop=(dc == DC - 1),
                        )
                    nc.vector.tensor_copy(dst[:, c, :], pps)

            # Natural-layout bf16 slices of wq/wk for the PE-offloaded
            # broadcast-add chunks (c in {6, 7}).
            PE_CHUNKS = (6, 7)
            wqn = sb_pool.tile([P, len(PE_CHUNKS), P], bf16)
            wkn = sb_pool.tile([P, len(PE_CHUNKS), P], bf16)
            for j, c in enumerate(PE_CHUNKS):
                tq = ps_pool.tile([P, P], f32, tag="tp")
                nc.tensor.transpose(tq, wq[:, c, :], ident)
                nc.scalar.copy(wqn[:, j, :], tq)
                tk = ps_pool.tile([P, P], bf16, tag="tp")
                nc.tensor.transpose(tk, wk[:, c, :], ident_bf)
                nc.scalar.copy(wkn[:, j, :], tk)

            # Main loop: scores_ps[s, t] accumulated over u-chunks.
            scores_ps = acc_ps_pool.tile([P, Tq], f32, tag="scores")
            NDV = UC - len(PE_CHUNKS)  # chunks handled by DVE adds
            for g in range(Tq // G_T):
                t0 = g * G_T
                stage = stage_pool.tile([P, G_T, NDV * P], bf16, tag="stage")
                mt = m_pool.tile([P, G_T, UC, P], bf16, tag="mt")
                for tl in range(G_T):
                    t = t0 + tl
                    for c in range(NDV):
                        nc.vector.tensor_scalar_add(
                            out=stage[:, tl, c * P:(c + 1) * P],
                            in0=wk[:, c, :],
                            scalar1=wq[:, c, t:t + 1],
                        )
                nc.scalar.activation(mt[:, :, :NDV, :], stage, AF.Tanh)
                for j, c in enumerate(PE_CHUNKS):
                    stg = stg_ps_pool.tile([P, G_T, P], f32, tag="stg")
                    nc.tensor.matmul(
                        stg[:],
                        lhsT=wqn[:, j, :],
                        rhs=ident_bf[:, t0:t0 + G_T, None].to_broadcast((P, G_T, P)),
                        start=True, stop=False,
                    )
                    nc.tensor.matmul(
                        stg[:],
                        lhsT=wkn[:, j, :],
                        rhs=oh_s[:],
                        start=False, stop=True,
                    )
                    nc.scalar.activation(mt[:, :, c, :], stg, AF.Tanh)
                for tl in range(G_T):
                    t = t0 + tl
                    for c in range(UC):
                        nc.tensor.matmul(
                            scores_ps[:, t:t + 1],
                            lhsT=mt[:, tl, c, :],
                            rhs=scale_bf[:, c:c + 1],
                            start=(c == 0),
                            stop=(c == UC - 1),
                        )

            # Softmax + context in two t-halves so the first half overlaps
            # the tail of the main loop. scores_ps is laid out [s, t].
            HT = Tq // 2
            for h in range(2):
                hs = h * HT
                sc_h = misc_pool.tile([P, HT], f32, tag="sc_h")
                nc.vector.tensor_scalar_add(
                    sc_h, scores_ps[:, hs:hs + HT], mask_neg)
                tsp = ps_pool.tile([HT, P], f32, tag="tp")
                nc.tensor.transpose(tsp, sc_h, ident)
                neg_mx = misc_pool.tile([HT, 1], f32, tag="mx")
                nc.vector.reduce_max(
                    neg_mx, tsp, axis=mybir.AxisListType.X, negate=True)
                p_sb = misc_pool.tile([HT, Tv], f32, tag="p_h")
                sum_sb = misc_pool.tile([HT, 1], f32, tag="sum_h")
                nc.scalar.activation(p_sb, tsp, AF.Exp, bias=neg_mx,
                                     accum_out=sum_sb)
                rsum = misc_pool.tile([HT, 1], f32, tag="rs_h")
                nc.vector.reciprocal(rsum, sum_sb)
                attn_sb = misc_pool.tile([HT, Tv], f32, tag="attn_h")
                nc.vector.tensor_scalar_mul(attn_sb, p_sb, rsum)
                nc.sync.dma_start(attn_d[hs:hs + HT, :], attn_sb[:])
                atp = ps_pool.tile([P, HT], f32, tag="tp")
                nc.tensor.transpose(atp, attn_sb, ident[:HT, :HT])
                attnT = misc_pool.tile([P, HT], f32, tag="attnT_h")
                nc.scalar.copy(attnT, atp)
                ctx_ps = acc_ps_pool.tile([HT, D], f32, tag="ctx")
                nc.tensor.matmul(ctx_ps, lhsT=attnT, rhs=v_sb,
                                 start=True, stop=True)
                ctx_sb = misc_pool.tile([HT, D], f32, tag="ctx_h")
                nc.vector.tensor_copy(ctx_sb, ctx_ps)
                nc.sync.dma_start(ctx_d[hs:hs + HT, :], ctx_sb[:])

    return nc


def get_bass():
    if "nc" not in _CACHE:
        nc = _build_bass()
        # Run the bacc passes (wait splitting, register allocation, act table
        # loads) and freeze before the PJRT lowering serializes the module.
        nc.finalize()
        _CACHE["nc"] = nc
    return _CACHE["nc"]


def make_in_maps(query, value, mask, W1, W2, scale):
    query = np.ascontiguousarray(np.asarray(query, dtype=np.float32))
    value = np.ascontiguousarray(np.asarray(value, dtype=np.float32))
    mask = np.asarray(mask)
    W1 = np.ascontiguousarray(np.asarray(W1, dtype=np.float32))
    W2 = np.ascontiguousarray(np.asarray(W2, dtype=np.float32))
    scale = np.asarray(scale, dtype=np.float32)
    # scale_pc[p, c] = scale[c*128 + p]
    scale_pc = np.ascontiguousarray(scale.reshape(UC, P).T)
    in_maps = []
    for b in range(B):
        in_maps.append({
            "q": np.ascontiguousarray(query[b]),
            "v": np.ascontiguousarray(value[b]),
            "w1": W1,
            "w2": W2,
            "scale_pc": scale_pc,
            "mask_p": np.ascontiguousarray(
                mask[b].astype(np.uint8).reshape(P, 1)),
        })
    return in_maps


def run(query, value, mask, W1, W2, scale, trace=False):
    from concourse.bass_utils import run_bass_kernel_spmd

    nc = get_bass()
    in_maps = make_in_maps(query, value, mask, W1, W2, scale)
    res = run_bass_kernel_spmd(nc, in_maps, list(range(NCORES)), trace=trace)
    ctx = np.stack([res.results[b]["ctx"] for b in range(B)])
    attn = np.stack([res.results[b]["attn"] for b in range(B)])
    return (ctx, attn), res


def kernel(**inputs):
    (ctx, attn), _ = run(
        inputs["query"], inputs["value"], inputs["mask"],
        inputs["W1"], inputs["W2"], inputs["scale"],
    )
    return ctx, attn


# revision 17
# speedup vs baseline: 3.6524x; 1.0085x over previous
"""Bahdanau (additive) attention kernel for Trainium2, SPMD over 8 NeuronCores.

Problem shapes (hardcoded): B=8, Tq=Tv=128, D=512, U=1024, fp32.
Sharding: data-parallel over batch -- one batch element per core; W1/W2/scale
replicated.

Per-core algorithm:
  wq = q @ W1            [Tq, U]   (kept transposed: [u_part, t])
  wk = v @ W2            [Tv, U]   (kept transposed: [u_part, s])
  scores[t, s] = sum_u scale[u] * tanh(wq[t, u] + wk[s, u])
     - broadcast-add on DVE (tensor_scalar, per-partition scalar = wq column)
     - tanh on ACT engine (big fused tiles)
     - reduction over u via PE matmuls (lhsT = tanh tile [u, s], rhs = scale
       column [u, 1]) accumulating into a PSUM tile laid out [s, t]
  softmax over s (key axis) with max-subtraction, on [t, s] layout
  context = attn @ v via one PE matmul (lhsT = attn^T [s, t], rhs = v [s, d])
"""

import numpy as np

B, Tq, Tv, D, U = 8, 128, 128, 512, 1024
P = 128
UC = U // P  # 8 u-chunks
DC = D // P  # 4 d-chunks
G_T = 4      # query rows per tanh group
NCORES = 8

_CACHE = {}


def _build_bass():
    import concourse.bass as bass
    import concourse.mybir as mybir
    from concourse import bacc
    import concourse.tile as tile
    from concourse.masks import make_identity

    f32 = mybir.dt.float32
    bf16 = mybir.dt.bfloat16
    u8 = mybir.dt.uint8
    AF = mybir.ActivationFunctionType

    nc = bacc.Bacc()

    q_d = nc.declare_dram_parameter("q", [Tq, D], f32, isOutput=False)
    v_d = nc.declare_dram_parameter("v", [Tv, D], f32, isOutput=False)
    w1_d = nc.declare_dram_parameter("w1", [D, U], f32, isOutput=False)
    w2_d = nc.declare_dram_parameter("w2", [D, U], f32, isOutput=False)
    sc_d = nc.declare_dram_parameter("scale_pc", [P, UC], f32, isOutput=False)
    mk_d = nc.declare_dram_parameter("mask_p", [P, 1], u8, isOutput=False)
    ctx_d = nc.declare_dram_parameter("ctx", [Tq, D], f32, isOutput=True)
    attn_d = nc.declare_dram_parameter("attn", [Tq, Tv], f32, isOutput=True)

    with tile.TileContext(nc) as tc:
        with (
            tc.tile_pool(name="const", bufs=1) as const_pool,
            tc.tile_pool(name="w", bufs=1) as w_pool,
            tc.tile_pool(name="sb", bufs=1) as sb_pool,
            tc.tile_pool(name="stage", bufs=6) as stage_pool,
            tc.tile_pool(name="mt", bufs=6) as m_pool,
            tc.tile_pool(name="misc", bufs=2) as misc_pool,
            tc.tile_pool(name="ps", bufs=2, space="PSUM") as ps_pool,
            tc.tile_pool(name="accps", bufs=1, space="PSUM") as acc_ps_pool,
            tc.tile_pool(name="stgps", bufs=2, space="PSUM") as stg_ps_pool,
        ):
            ident = const_pool.tile([P, P], f32)
            make_identity(nc, ident)
            ident_bf = const_pool.tile([P, P], bf16)
            make_identity(nc, ident_bf)
            oh_s = const_pool.tile([P, 4, P], bf16)
            for tl in range(4):
                nc.vector.tensor_copy(oh_s[:, tl, :], ident_bf[:])

            # Warm the ACT table set (exp_and_others contains tanh+exp) ASAP
            # so the ~2.7us table load overlaps the input DMAs.
            dummy = const_pool.tile([P, 1], f32)
            nc.vector.memset(dummy, 0.0)
            nc.scalar.activation(dummy, dummy, AF.Tanh)

            scale_sb = const_pool.tile([P, UC], f32)
            nc.sync.dma_start(scale_sb[:], sc_d[:])
            scale_bf = const_pool.tile([P, UC], bf16)
            nc.vector.tensor_copy(scale_bf[:], scale_sb[:])
            mask_u8 = const_pool.tile([P, 1], u8)
            nc.sync.dma_start(mask_u8[:], mk_d[:])
            # mask_neg[s] = (mask[s] - 1) * 1e9  -> 0 where valid, -1e9 where masked
            mask_neg = const_pool.tile([P, 1], f32)
            nc.vector.tensor_scalar(
                out=mask_neg, in0=mask_u8, scalar1=1.0, scalar2=1e9,
                op0=mybir.AluOpType.subtract, op1=mybir.AluOpType.mult,
            )

            q_sb = sb_pool.tile([P, D], f32)
            nc.sync.dma_start(q_sb[:], q_d[:])
            v_sb = sb_pool.tile([P, D], f32)
            nc.sync.dma_start(v_sb[:], v_d[:])
            w1_sb = w_pool.tile([P, DC, U], f32)
            w2_sb = w_pool.tile([P, DC, U], f32)
            w1_r = w1_d.rearrange("(dc p) u -> p dc u", p=P)
            w2_r = w2_d.rearrange("(dc p) u -> p dc u", p=P)
            H = U // 2
            nc.sync.dma_start(w2_sb[:, :, :H], w2_r[:, :, :H])
            nc.sync.dma_start(w1_sb[:, :, :H], w1_r[:, :, :H])
            nc.sync.dma_start(w2_sb[:, :, H:], w2_r[:, :, H:])
            nc.sync.dma_start(w1_sb[:, :, H:], w1_r[:, :, H:])

            # Transpose q, v to [d_part, t/s] chunks for the projection matmuls.
            qT = sb_pool.tile([P, DC, P], f32)
            vT = sb_pool.tile([P, DC, P], f32)
            for src, dst in ((q_sb, qT), (v_sb, vT)):
                for dc in range(DC):
                    tps = ps_pool.tile([P, P], f32, tag="tp")
                    nc.tensor.transpose(tps, src[:, dc * P:(dc + 1) * P], ident)
                    nc.vector.tensor_copy(dst[:, dc, :], tps)

            # Projections: wq[p, c, t] = sum_d W1[d, c*128+p] * q[t, d]
            wq = sb_pool.tile([P, UC, P], f32)
            wk = sb_pool.tile([P, UC, P], bf16)
            for c in range(UC):
                for wsrc, xT, dst in ((w2_sb, vT, wk), (w1_sb, qT, wq)):
                    pps = ps_pool.tile([P, P], f32, tag="proj")
                    for dc in range(DC):
                        nc.tensor.matmul(
                            pps,
                            lhsT=wsrc[:, dc, c * P:(c + 1) * P],
                            rhs=xT[:, dc, :],
                            start=(dc == 0),
                            stop=(dc == DC - 1),
                        )
                    nc.vector.tensor_copy(dst[:, c, :], pps)

            # Natural-layout bf16 slices of wq/wk for the PE-offloaded
            # broadcast-add chunks (c in {6, 7}).
            PE_CHUNKS = (6, 7)
            wqn = sb_pool.tile([P, len(PE_CHUNKS), P], bf16)
            wkn = sb_pool.tile([P, len(PE_CHUNKS), P], bf16)
            for j, c in enumerate(PE_CHUNKS):
                tq = ps_pool.tile([P, P], f32, tag="tp")
                nc.tensor.transpose(tq, wq[:, c, :], ident)
                nc.vector.tensor_copy(wqn[:, j, :], tq)
                tk = ps_pool.tile([P, P], bf16, tag="tp")
                nc.tensor.transpose(tk, wk[:, c, :], ident_bf)
                nc.vector.tensor_copy(wkn[:, j, :], tk)

            # Main loop: scores_ps[s, t] accumulated over u-chunks.
            scores_ps = acc_ps_pool.tile([P, Tq], f32, tag="scores")
            NDV = UC - len(PE_CHUNKS)  # chunks handled by DVE adds
            for g in range(Tq // G_T):
                t0 = g * G_T
                stage = stage_pool.tile([P, G_T, NDV * P], bf16, tag="stage")
                mt = m_pool.tile([P, G_T, UC, P], bf16, tag="mt")
                for tl in range(G_T):
                    t = t0 + tl
                    for c in range(NDV):
                        nc.vector.tensor_scalar_add(
                            out=stage[:, tl, c * P:(c + 1) * P],
                            in0=wk[:, c, :],
                            scalar1=wq[:, c, t:t + 1],
                        )
                nc.scalar.activation(mt[:, :, :NDV, :], stage, AF.Tanh)
                for j, c in enumerate(PE_CHUNKS):
                    stg = stg_ps_pool.tile([P, G_T, P], f32, tag="stg")
                    nc.tensor.matmul(
                        stg[:],
                        lhsT=wqn[:, j, :],
                        rhs=ident_bf[:, t0:t0 + G_T, None].to_broadcast((P, G_T, P)),
                        start=True, stop=False,
                    )
                    nc.tensor.matmul(
                        stg[:],
                        lhsT=wkn[:, j, :],
                        rhs=oh_s[:],
                        start=False, stop=True,
                    )
                    nc.scalar.activation(mt[:, :, c, :], stg, AF.Tanh)
                for tl in range(G_T):
                    t = t0 + tl
                    for c in range(UC):
                        nc.tensor.matmul(
                            scores_ps[:, t:t + 1],
                            lhsT=mt[:, tl, c, :],
                            rhs=scale_bf[:, c:c + 1],
                            start=(c == 0),
                            stop=(c == UC - 1),
                        )

            # Softmax + context in two t-halves so the first half overlaps
            # the tail of the main loop. scores_ps is laid out [s, t].
            HT = Tq // 2
            for h in range(2):
                hs = h * HT
                sc_h = misc_pool.tile([P, HT], f32, tag="sc_h")
                nc.vector.tensor_scalar_add(
                    sc_h, scores_ps[:, hs:hs + HT], mask_neg)
                tsp = ps_pool.tile([HT, P], f32, tag="tp")
                nc.tensor.transpose(tsp, sc_h, ident)
                neg_mx = misc_pool.tile([HT, 1], f32, tag="mx")
                nc.vector.reduce_max(
                    neg_mx, tsp, axis=mybir.AxisListType.X, negate=True)
                p_sb = misc_pool.tile([HT, Tv], f32, tag="p_h")
                sum_sb = misc_pool.tile([HT, 1], f32, tag="sum_h")
                nc.scalar.activation(p_sb, tsp, AF.Exp, bias=neg_mx,
                                     accum_out=sum_sb)
                rsum = misc_pool.tile([HT, 1], f32, tag="rs_h")
                nc.vector.reciprocal(rsum, sum_sb)
                attn_sb = misc_pool.tile([HT, Tv], f32, tag="attn_h")
                nc.vector.tensor_scalar_mul(attn_sb, p_sb, rsum)
                nc.sync.dma_start(attn_d[hs:hs + HT, :], attn_sb[:])
                atp = ps_pool.tile([P, HT], f32, tag="tp")
                nc.tensor.transpose(atp, attn_sb, ident[:HT, :HT])
                attnT = misc_pool.tile([P, HT], f32, tag="attnT_h")
                nc.scalar.copy(attnT, atp)
                ctx_ps = acc_ps_pool.tile([HT, D], f32, tag="ctx")
                nc.tensor.matmul(ctx_ps, lhsT=attnT, rhs=v_sb,
                                 start=True, stop=True)
                ctx_sb = misc_pool.tile([HT, D], f32, tag="ctx_h")
                nc.vector.tensor_copy(ctx_sb, ctx_ps)
                nc.sync.dma_start(ctx_d[hs:hs + HT, :], ctx_sb[:])

    return nc


def get_bass():
    if "nc" not in _CACHE:
        nc = _build_bass()
        # Run the bacc passes (wait splitting, register allocation, act table
        # loads) and freeze before the PJRT lowering serializes the module.
        nc.finalize()
        _CACHE["nc"] = nc
    return _CACHE["nc"]


def make_in_maps(query, value, mask, W1, W2, scale):
    query = np.ascontiguousarray(np.asarray(query, dtype=np.float32))
    value = np.ascontiguousarray(np.asarray(value, dtype=np.float32))
    mask = np.asarray(mask)
    W1 = np.ascontiguousarray(np.asarray(W1, dtype=np.float32))
    W2 = np.ascontiguousarray(np.asarray(W2, dtype=np.float32))
    scale = np.asarray(scale, dtype=np.float32)
    # scale_pc[p, c] = scale[c*128 + p]
    scale_pc = np.ascontiguousarray(scale.reshape(UC, P).T)
    in_maps = []
    for b in range(B):
        in_maps.append({
            "q": np.ascontiguousarray(query[b]),
            "v": np.ascontiguousarray(value[b]),
            "w1": W1,
            "w2": W2,
            "scale_pc": scale_pc,
            "mask_p": np.ascontiguousarray(
                mask[b].astype(np.uint8).reshape(P, 1)),
        })
    return in_maps


def run(query, value, mask, W1, W2, scale, trace=False):
    from concourse.bass_utils import run_bass_kernel_spmd

    nc = get_bass()
    in_maps = make_in_maps(query, value, mask, W1, W2, scale)
    res = run_bass_kernel_spmd(nc, in_maps, list(range(NCORES)), trace=trace)
    ctx = np.stack([res.results[b]["ctx"] for b in range(B)])
    attn = np.stack([res.results[b]["attn"] for b in range(B)])
    return (ctx, attn), res


def kernel(**inputs):
    (ctx, attn), _ = run(
        inputs["query"], inputs["value"], inputs["mask"],
        inputs["W1"], inputs["W2"], inputs["scale"],
    )
    return ctx, attn
